# revision 6
# baseline (speedup 1.0000x reference)
"""Trainium2 Bass kernel for nn_CRSDCell_71339406786971.

kernel(**inputs) takes the FULL (unsharded) numpy inputs and returns the full
(h_new, r0_out, r1_out, r2_out) tuple. Internally: data-parallel shard of the
batch dim across 8 NeuronCores, replicated weights, on-chip AllReduce of the
batch-mean key/value and the Hebbian outer product.
"""
import sys
import numpy as np

try:
    import concourse.bass as bass  # noqa: F401
except Exception:
    sys.path.insert(0, "/opt/trn_rl_repo")

import concourse.bass as bass  # noqa: F811
import concourse.mybir as mybir
import concourse.tile as tile
from concourse import bacc
from concourse.bass_utils import run_bass_kernel_spmd
from concourse.masks import make_identity

F32 = mybir.dt.float32
F32R = mybir.dt.float32r
BF16 = mybir.dt.bfloat16
AF = mybir.ActivationFunctionType
ALU = mybir.AluOpType

NCORES = 8
B = 65536
BC = B // NCORES          # 8192 rows per core
TB = 512                  # batch tile (free dim per matmul)
NT = BC // TB             # 16 tiles per core
D_X, D_H = 256, 512
RES = 512                 # total reservoir dim (128+128+256)
D_K = D_V = 64
SLOTS = 64
HEBB_DECAY, HEBB_ETA = 0.9, 0.1

# module-level knobs for the test harness
TRACE = False
LAST_EXEC_NS = None
LAST_RESULTS = None

_AR_GROUPS = [list(range(NCORES))]


def _build():
    nc = bacc.Bacc("TRN2", target_bir_lowering=False, debug=False,
                   num_devices=NCORES)

    d = {}
    # ---- DRAM I/O ----
    d["x_d"] = nc.dram_tensor("x", [BC, D_X], F32, kind="ExternalInput")
    d["h_d"] = nc.dram_tensor("h_prev", [BC, D_H], F32, kind="ExternalInput")
    # r inputs are pre-scaled by a=sigmoid(res_logit_alpha) on host
    d["r0_d"] = nc.dram_tensor("ar0", [BC, 128], F32, kind="ExternalInput")
    d["r1_d"] = nc.dram_tensor("ar1", [BC, 128], F32, kind="ExternalInput")
    d["r2_d"] = nc.dram_tensor("ar2", [BC, 256], F32, kind="ExternalInput")

    d["wx_d"] = nc.dram_tensor("wxT", [D_X, RES], F32R, kind="ExternalInput")
    d["wh_d"] = nc.dram_tensor("whT", [D_H, RES], F32R, kind="ExternalInput")
    d["at_d"] = nc.dram_tensor("aT", [D_H, D_H], F32R, kind="ExternalInput")
    d["bt_d"] = nc.dram_tensor("bT", [RES, D_H], F32R, kind="ExternalInput")
    d["ut_d"] = nc.dram_tensor("uT", [D_X, D_H], F32R, kind="ExternalInput")
    d["key_d"] = nc.dram_tensor("keyT", [RES + D_X, D_K], F32R,
                                kind="ExternalInput")
    d["val_d"] = nc.dram_tensor("valT", [D_H, D_V], F32R, kind="ExternalInput")
    d["gate_d"] = nc.dram_tensor("gateT", [D_H + RES + D_K + D_V, D_H], BF16,
                                 kind="ExternalInput")
    d["rm_d"] = nc.dram_tensor("rmT", [D_V, D_H], F32R, kind="ExternalInput")
    d["kt0_d"] = nc.dram_tensor("ktT", [D_K, SLOTS], F32R, kind="ExternalInput")
    d["v0_d"] = nc.dram_tensor("v0", [SLOTS, D_V], F32R, kind="ExternalInput")
    d["hebb_d"] = nc.dram_tensor("hebb45", [D_K, D_V], F32,
                                 kind="ExternalInput")

    d["br_d"] = nc.dram_tensor("br", [RES], F32, kind="ExternalInput")
    d["oma_d"] = nc.dram_tensor("oma", [RES], F32, kind="ExternalInput")
    d["ub_d"] = nc.dram_tensor("ub", [D_H], F32, kind="ExternalInput")
    d["keyb_d"] = nc.dram_tensor("keyb", [D_K], F32, kind="ExternalInput")
    d["valb_d"] = nc.dram_tensor("valb", [D_V], F32, kind="ExternalInput")
    d["gateb_d"] = nc.dram_tensor("gateb", [D_H], F32, kind="ExternalInput")
    d["rmb_d"] = nc.dram_tensor("rmb", [D_H], F32, kind="ExternalInput")

    d["hn_d"] = nc.dram_tensor("h_new", [BC, D_H], F32, kind="ExternalOutput")
    d["r0o_d"] = nc.dram_tensor("r0o", [BC, 128], F32, kind="ExternalOutput")
    d["r1o_d"] = nc.dram_tensor("r1o", [BC, 128], F32, kind="ExternalOutput")
    d["r2o_d"] = nc.dram_tensor("r2o", [BC, 256], F32, kind="ExternalOutput")

    with tile.TileContext(nc) as tc:
        with nc.allow_low_precision("f32r rounding of matmul inputs by design"):
            _emit(nc, tc, d)
    nc.compile()
    return nc


def _emit(nc, tc, d):
    from contextlib import ExitStack
    es = ExitStack()
    # pools that live for the whole kernel
    wpool = es.enter_context(tc.tile_pool(name="w", bufs=1))
    kpool = es.enter_context(tc.tile_pool(name="kres", bufs=1))
    opool = es.enter_context(tc.tile_pool(name="obm", bufs=3))
    ps_tr = es.enter_context(tc.tile_pool(name="ptr", bufs=2, space="PSUM"))
    ps_mm = es.enter_context(tc.tile_pool(name="pmm", bufs=2, space="PSUM"))
    ps_kv = es.enter_context(tc.tile_pool(name="pkv", bufs=1, space="PSUM"))
    ps_b = es.enter_context(tc.tile_pool(name="pb", bufs=3, space="PSUM"))
    dpool = es.enter_context(tc.tile_pool(name="dram", bufs=1, space="DRAM"))

    # ---- weights into SBUF ----
    def ldw(dram, ktot, m, dt, tag):
        kc = ktot // 128
        t = wpool.tile([128, kc * m], dt, tag=tag)
        nc.sync.dma_start(
            t[:].rearrange("p (c m) -> p c m", c=kc),
            dram[:].rearrange("(c p) m -> p c m", p=128))
        return t

    wx_s = ldw(d["wx_d"], D_X, RES, F32R, "wx")       # [128, 2*512]
    wh_s = ldw(d["wh_d"], D_H, RES, F32R, "wh")       # [128, 4*512]
    at_s = ldw(d["at_d"], D_H, D_H, F32R, "at")
    bt_s = ldw(d["bt_d"], RES, D_H, F32R, "bt")
    ut_s = ldw(d["ut_d"], D_X, D_H, F32R, "ut")
    key_s = ldw(d["key_d"], RES + D_X, D_K, F32R, "key")   # [128, 6*64]
    val_s = ldw(d["val_d"], D_H, D_V, F32R, "val")         # [128, 4*64]
    gate_s = ldw(d["gate_d"], D_H + RES + D_K + D_V, D_H, BF16, "gate")
    rm_s = wpool.tile([D_V, D_H], F32R)
    nc.sync.dma_start(rm_s[:], d["rm_d"][:])
    kt0_s = wpool.tile([D_K, SLOTS], F32R)
    nc.sync.dma_start(kt0_s[:], d["kt0_d"][:])
    v0_s = wpool.tile([SLOTS, D_V], F32R)
    nc.sync.dma_start(v0_s[:], d["v0_d"][:])
    hebb_s = wpool.tile([D_K, D_V], F32)
    nc.sync.dma_start(hebb_s[:], d["hebb_d"][:])

    def ldb(dram, n, tag):
        mc = n // 128
        t = wpool.tile([128, mc], F32, tag=tag)
        nc.sync.dma_start(t[:].rearrange("p (c o) -> p c o", o=1),
                          dram[:].rearrange("(c p) -> p c", p=128))
        return t

    br_s = ldb(d["br_d"], RES, "br")
    oma_s = ldb(d["oma_d"], RES, "oma")
    ub_s = ldb(d["ub_d"], D_H, "ub")
    gateb_s = ldb(d["gateb_d"], D_H, "gateb")
    rmb_s = ldb(d["rmb_d"], D_H, "rmb")
    keyb_s = wpool.tile([D_K, 1], F32)
    nc.sync.dma_start(keyb_s[:], d["keyb_d"][:].rearrange("(p o) -> p o", o=1))
    valb_s = wpool.tile([D_V, 1], F32)
    nc.sync.dma_start(valb_s[:], d["valb_d"][:].rearrange("(p o) -> p o", o=1))

    ident = wpool.tile([128, 128], F32)
    make_identity(nc, ident[:])
    ones_f = wpool.tile([64, 64], F32)
    nc.vector.memset(ones_f[:], 1.0)
    ones_r = wpool.tile([64, 64], F32R)
    nc.vector.tensor_copy(ones_r[:], ones_f[:])
    nones_f = wpool.tile([1, 64], F32)
    nc.vector.memset(nones_f[:], -1.0)
    nones_r = wpool.tile([1, 64], F32R)
    nc.vector.tensor_copy(nones_r[:], nones_f[:])
    nh_f = wpool.tile([1, 64], F32)
    nc.vector.memset(nh_f[:], -0.5)
    nh_r = wpool.tile([1, 64], F32R)
    nc.vector.tensor_copy(nh_r[:], nh_f[:])

    # k kept resident across both phases
    k_res = kpool.tile([D_K, BC], F32R)
    k_acc = wpool.tile([D_K, 1], F32)
    v_acc = wpool.tile([D_V, 1], F32)
    nc.vector.memset(k_acc[:], 0.0)
    nc.vector.memset(v_acc[:], 0.0)

    # DRAM staging, one tile per batch-tile for fine-grained A->B deps.
    # Layout [128, fc*TB + col] (feature-chunk major within each tile).
    rfm_ds = [dpool.tile([128, 4 * TB], BF16, tag=f"rfm{t}", name=f"rfm{t}") for t in range(NT)]
    hpfm_ds = [dpool.tile([128, 4 * TB], BF16, tag=f"hpfm{t}", name=f"hpfm{t}") for t in range(NT)]
    htfm_ds = [dpool.tile([128, 4 * TB], F32, tag=f"htfm{t}", name=f"htfm{t}") for t in range(NT)]
    red_in = dpool.tile([66, 64], F32)
    red_out = dpool.tile([66, 64], F32)

    kv_ps = ps_kv.tile([D_K, D_V], F32)         # persistent outer-product acc

    # ================= PHASE A =================
    esA = ExitStack()
    lpool = esA.enter_context(tc.tile_pool(name="loads", bufs=2))
    fmpool = esA.enter_context(tc.tile_pool(name="fm", bufs=2))
    ropool = esA.enter_context(tc.tile_pool(name="rout", bufs=6))
    htpool = esA.enter_context(tc.tile_pool(name="ht", bufs=5))
    apool = esA.enter_context(tc.tile_pool(name="smallA", bufs=2))

    for t in range(NT):
        b0 = t * TB
        # -- loads (batch-major) --
        x_l = lpool.tile([128, 4 * D_X], F32, tag="xl")
        nc.sync.dma_start(
            x_l[:].rearrange("p (c dd) -> p c dd", c=4),
            d["x_d"][b0:b0 + TB].rearrange("(c p) dd -> p c dd", p=128))
        h_l = lpool.tile([128, 4 * D_H], F32, tag="hl")
        nc.sync.dma_start(
            h_l[:].rearrange("p (c dd) -> p c dd", c=4),
            d["h_d"][b0:b0 + TB].rearrange("(c p) dd -> p c dd", p=128))
        r_l = lpool.tile([128, 4 * RES], F32, tag="rl")
        for rd_, off, w in ((d["r0_d"], 0, 128), (d["r1_d"], 128, 128),
                            (d["r2_d"], 256, 256)):
            nc.sync.dma_start(
                r_l[:].rearrange("p (c dd) -> p c dd", c=4)[:, :, off:off + w],
                rd_[b0:b0 + TB].rearrange("(c p) dd -> p c dd", p=128))

        x3 = x_l[:].rearrange("p (c dd) -> p c dd", c=4)
        h3 = h_l[:].rearrange("p (c dd) -> p c dd", c=4)
        r3 = r_l[:].rearrange("p (c dd) -> p c dd", c=4)

        # -- transpose x, h to feature-major (f32r via evac) --
        xT = fmpool.tile([128, 2 * TB], F32R, tag="xT")
        for fc in range(2):
            ps = ps_tr.tile([128, TB], F32, tag="trA")
            for c in range(4):
                nc.tensor.transpose(ps[:, c * 128:(c + 1) * 128],
                                    x3[:, c, fc * 128:(fc + 1) * 128], ident[:])
            if fc == 0:
                nc.scalar.copy(xT[:, fc * TB:(fc + 1) * TB], ps[:])
            else:
                nc.vector.tensor_copy(xT[:, fc * TB:(fc + 1) * TB], ps[:])
        hT = fmpool.tile([128, 4 * TB], F32R, tag="hT")
        for fc in range(4):
            ps = ps_tr.tile([128, TB], F32, tag="trA")
            for c in range(4):
                nc.tensor.transpose(ps[:, c * 128:(c + 1) * 128],
                                    h3[:, c, fc * 128:(fc + 1) * 128], ident[:])
            if fc % 2 == 0:
                nc.scalar.copy(hT[:, fc * TB:(fc + 1) * TB], ps[:])
            else:
                nc.vector.tensor_copy(hT[:, fc * TB:(fc + 1) * TB], ps[:])
        # stage h_prev fm as bf16 (cast on SWDGE dma)
        nc.gpsimd.dma_start(hpfm_ds[t][:], hT[:].bitcast(F32))

        # -- reservoir update per output chunk --
        r_outs = []
        for mc in range(4):
            ps_r = ps_tr.tile([128, TB], F32, tag="trA")
            for c in range(4):
                nc.tensor.transpose(ps_r[:, c * 128:(c + 1) * 128],
                                    r3[:, c, mc * 128:(mc + 1) * 128], ident[:])
            pm = ps_mm.tile([128, TB], F32, tag="mm")
            nmm = 6
            i = 0
            for kc in range(2):
                nc.tensor.matmul(pm[:], wx_s[:, kc * RES + mc * 128:
                                             kc * RES + (mc + 1) * 128],
                                 xT[:, kc * TB:(kc + 1) * TB],
                                 start=(i == 0), stop=(i == nmm - 1)); i += 1
            for kc in range(4):
                nc.tensor.matmul(pm[:], wh_s[:, kc * RES + mc * 128:
                                             kc * RES + (mc + 1) * 128],
                                 hT[:, kc * TB:(kc + 1) * TB],
                                 start=(i == 0), stop=(i == nmm - 1)); i += 1
            th = apool.tile([128, TB], F32, tag="th")
            nc.scalar.activation(th[:], pm[:], AF.Tanh, bias=br_s[:, mc:mc + 1])
            ro = ropool.tile([128, TB], F32R, tag="ro")
            # r_out = (tanh * (1-a)) + a*r_prev  (a*r_prev transposed, in PSUM)
            nc.vector.scalar_tensor_tensor(ro[:], th[:], oma_s[:, mc:mc + 1],
                                           ps_r[:], ALU.mult, ALU.add)
            r_outs.append(ro)
            # stage r_cat fm as bf16
            nc.gpsimd.dma_start(rfm_ds[t][:, mc * TB:(mc + 1) * TB],
                                ro[:].bitcast(F32))

        # -- h_tilde --
        ht_outs = []
        for mc in range(4):
            pm = ps_mm.tile([128, TB], F32, tag="mm")
            nmm = 10
            i = 0
            for kc in range(4):
                nc.tensor.matmul(pm[:], at_s[:, kc * D_H + mc * 128:
                                             kc * D_H + (mc + 1) * 128],
                                 hT[:, kc * TB:(kc + 1) * TB],
                                 start=(i == 0), stop=(i == nmm - 1)); i += 1
            for kc in range(4):
                nc.tensor.matmul(pm[:], bt_s[:, kc * D_H + mc * 128:
                                             kc * D_H + (mc + 1) * 128],
                                 r_outs[kc][:],
                                 start=(i == 0), stop=(i == nmm - 1)); i += 1
            for kc in range(2):
                nc.tensor.matmul(pm[:], ut_s[:, kc * D_H + mc * 128:
                                             kc * D_H + (mc + 1) * 128],
                                 xT[:, kc * TB:(kc + 1) * TB],
                                 start=(i == 0), stop=(i == nmm - 1)); i += 1
            ht = htpool.tile([128, TB], F32R, tag="htt")
            nc.scalar.activation(ht[:], pm[:], AF.Gelu, bias=ub_s[:, mc:mc + 1])
            ht_outs.append(ht)
            nc.sync.dma_start(htfm_ds[t][:, mc * TB:(mc + 1) * TB],
                               ht[:].bitcast(F32))

        # -- k and v (with running batch sums) --
        pk = ps_mm.tile([64, TB], F32, tag="mm")
        for i in range(6):
            rhs = r_outs[i][:] if i < 4 else xT[:, (i - 4) * TB:(i - 3) * TB]
            nc.tensor.matmul(pk[:], key_s[:, i * D_K:(i + 1) * D_K], rhs,
                             start=(i == 0), stop=(i == 5))
        kp = apool.tile([D_K, 1], F32, tag="kp")
        nc.scalar.activation(k_res[:, b0:b0 + TB], pk[:], AF.Identity,
                             bias=keyb_s[:, :], accum_out=kp[:])
        nc.vector.tensor_add(k_acc[:], k_acc[:], kp[:])

        pv = ps_mm.tile([64, TB], F32, tag="mm")
        for i in range(4):
            nc.tensor.matmul(pv[:], val_s[:, i * D_V:(i + 1) * D_V],
                             ht_outs[i][:], start=(i == 0), stop=(i == 3))
        v_t = apool.tile([D_V, TB], F32, tag="vt")
        vp = apool.tile([D_V, 1], F32, tag="vp")
        nc.scalar.activation(v_t[:], pv[:], AF.Identity, bias=valb_s[:, :],
                             accum_out=vp[:])
        nc.vector.tensor_add(v_acc[:], v_acc[:], vp[:])

        # -- kv outer product (exact fp32) --
        pt = ps_tr.tile([128, TB], F32, tag="trA")
        for i in range(4):
            nc.tensor.transpose(
                pt[:, i * 64:(i + 1) * 64],
                k_res[:, b0 + i * 128:b0 + (i + 1) * 128].bitcast(F32),
                ident[0:64, 0:64])
            nc.tensor.transpose(pt[:, 256 + i * 64:256 + (i + 1) * 64],
                                v_t[:, i * 128:(i + 1) * 128],
                                ident[0:64, 0:64])
        kvT = apool.tile([128, TB], F32, tag="kvT")
        nc.vector.tensor_copy(kvT[:], pt[:])
        for i in range(4):
            nc.tensor.matmul(kv_ps[:], kvT[:, i * 64:(i + 1) * 64],
                             kvT[:, 256 + i * 64:256 + (i + 1) * 64],
                             start=(t == 0 and i == 0),
                             stop=(t == NT - 1 and i == 3))

        # -- res outputs: transpose back to batch-major, store --
        for c in range(4):
            po = ps_tr.tile([128, TB], F32, tag="trA")
            for mc in range(4):
                nc.tensor.transpose(
                    po[:, mc * 128:(mc + 1) * 128],
                    r_outs[mc][:, c * 128:(c + 1) * 128].bitcast(F32),
                    ident[:])
            ob = opool.tile([128, TB], F32, tag="ob")
            if c % 2 == 0:
                nc.scalar.copy(ob[:], po[:])
            else:
                nc.vector.tensor_copy(ob[:], po[:])
            rb = b0 + c * 128
            nc.sync.dma_start(d["r0o_d"][rb:rb + 128, :], ob[:, 0:128])
            nc.sync.dma_start(d["r1o_d"][rb:rb + 128, :], ob[:, 128:256])
            nc.sync.dma_start(d["r2o_d"][rb:rb + 128, :], ob[:, 256:512])

    esA.close()

    # ================= ALLREDUCE =================
    kv_s = wpool.tile([D_K, D_V], F32)
    nc.vector.tensor_copy(kv_s[:], kv_ps[:])
    nc.sync.dma_start(red_in[0:64, :], kv_s[:])
    nc.sync.dma_start(red_in[64:65, :].rearrange("o (p q) -> p (o q)", q=1),
                      k_acc[:])
    nc.sync.dma_start(red_in[65:66, :].rearrange("o (p q) -> p (o q)", q=1),
                      v_acc[:])
    nc.gpsimd.collective_compute("AllReduce", ALU.add,
                                 replica_groups=_AR_GROUPS,
                                 ins=[red_in.opt()], outs=[red_out.opt()])
    kv_r = wpool.tile([D_K, D_V], F32)
    nc.sync.dma_start(kv_r[:], red_out[0:64, :])
    kmean = wpool.tile([D_K, 1], F32)
    nc.sync.dma_start(kmean[:], red_out[64:65, :].rearrange(
        "o (p q) -> p (o q)", q=1))
    vmean = wpool.tile([1, D_V], F32)
    nc.sync.dma_start(vmean[:], red_out[65:66, :])
    # M_half = 0.45*hebb + (0.05/B)*kv_sum
    m_s = wpool.tile([D_K, D_V], F32R)
    nc.vector.scalar_tensor_tensor(m_s[:], kv_r[:], 0.5 * HEBB_ETA / B,
                                   hebb_s[:], ALU.mult, ALU.add)
    # slot-0 key/value = batch means
    nc.vector.tensor_scalar_mul(kt0_s[:, 0:1], kmean[:], 1.0 / B)
    nc.vector.tensor_scalar_mul(v0_s[0:1, :], vmean[:], 1.0 / B)

    # ================= PHASE B =================
    esB = ExitStack()
    bpool = esB.enter_context(tc.tile_pool(name="bload", bufs=2))
    cpool = esB.enter_context(tc.tile_pool(name="smallB", bufs=2))
    gpool = esB.enter_context(tc.tile_pool(name="gs", bufs=2))
    hpool = esB.enter_context(tc.tile_pool(name="hn", bufs=6))

    for t in range(NT):
        b0 = t * TB
        rfm_t = bpool.tile([128, 4 * TB], BF16, tag="rfm")
        nc.sync.dma_start(rfm_t[:], rfm_ds[t][:])
        hpfm_t = bpool.tile([128, 4 * TB], BF16, tag="hpfm")
        nc.sync.dma_start(hpfm_t[:], hpfm_ds[t][:])
        ht_t = bpool.tile([128, 4 * TB], F32, tag="htl")
        nc.sync.dma_start(ht_t[:], htfm_ds[t][:])

        kk = k_res[:, b0:b0 + TB]

        # c = k / ||k||
        ksq = cpool.tile([D_K, TB], F32R, tag="ksq")
        nc.vector.tensor_mul(ksq[:], kk.bitcast(F32), kk.bitcast(F32))
        p_ss = ps_b.tile([1, TB], F32, tag="pb")
        nc.tensor.matmul(p_ss[:], ones_r[:, 0:1], ksq[:], start=True, stop=True)
        # 1/||k|| = exp(-0.5*ln(ssq)), broadcast via PE
        lss = cpool.tile([1, TB], F32R, tag="lss")
        nc.scalar.activation(lss[:], p_ss[:], AF.Ln)
        p_bc = ps_b.tile([64, TB], F32, tag="pb")
        nc.tensor.matmul(p_bc[:], nh_r[:], lss[:], start=True, stop=True)
        rsb = cpool.tile([64, TB], F32, tag="rsb")
        nc.scalar.activation(rsb[:], p_bc[:], AF.Exp)
        c_r = cpool.tile([D_K, TB], F32R, tag="cr")
        nc.vector.tensor_mul(c_r[:], kk.bitcast(F32), rsb[:])
        cv_bf = cpool.tile([128, TB], BF16, tag="cvbf")
        nc.vector.tensor_copy(cv_bf[0:64, :], c_r[:].bitcast(F32))

        # attention over slots (softmax denominators via PE ones-matmul)
        p_lg = ps_b.tile([SLOTS, TB], F32, tag="pb")
        nc.tensor.matmul(p_lg[:], kt0_s[:], c_r[:], start=True, stop=True)
        e_r = cpool.tile([SLOTS, TB], F32R, tag="er")
        nc.scalar.activation(e_r[:], p_lg[:], AF.Exp, scale=0.125)
        p_dn = ps_b.tile([1, TB], F32, tag="pb")
        nc.tensor.matmul(p_dn[:], ones_r[:, 0:1], e_r[:], start=True, stop=True)
        # 0.5/denom = exp(-ln(2*denom)), broadcast via PE
        ldn = cpool.tile([1, TB], F32R, tag="ldn")
        nc.scalar.activation(ldn[:], p_dn[:], AF.Ln, scale=2.0)
        p_rb = ps_b.tile([64, TB], F32, tag="pb")
        nc.tensor.matmul(p_rb[:], nones_r[:], ldn[:], start=True, stop=True)
        rdb = cpool.tile([64, TB], F32, tag="rdb")
        nc.scalar.activation(rdb[:], p_rb[:], AF.Exp)
        # e_n = e * (0.5/denom)  -> normalized halved attention weights
        e_n = cpool.tile([SLOTS, TB], F32R, tag="en")
        nc.vector.tensor_mul(e_n[:], e_r[:].bitcast(F32), rdb[:])
        # v_hat = V^T @ e_n + M_half^T @ c   (single PSUM accumulation)
        p_v = ps_b.tile([D_V, TB], F32, tag="pb")
        nc.tensor.matmul(p_v[:], v0_s[:], e_n[:], start=True, stop=False)
        nc.tensor.matmul(p_v[:], m_s[:], c_r[:], start=False, stop=True)
        vhat = cpool.tile([D_V, TB], F32R, tag="vhat")
        nc.vector.tensor_copy(vhat[:], p_v[:])
        nc.vector.tensor_copy(cv_bf[64:128, :], vhat[:].bitcast(F32))

        # gate + chi + h_new
        hn_outs = []
        for mc in range(4):
            pg = ps_mm.tile([128, TB], F32, tag="mm")
            i = 0
            for kc in range(4):
                nc.tensor.matmul(pg[:], gate_s[:, kc * D_H + mc * 128:
                                               kc * D_H + (mc + 1) * 128],
                                 hpfm_t[:, kc * TB:(kc + 1) * TB],
                                 start=(i == 0), stop=False); i += 1
            for kc in range(4):
                nc.tensor.matmul(pg[:], gate_s[:, (4 + kc) * D_H + mc * 128:
                                               (4 + kc) * D_H + (mc + 1) * 128],
                                 rfm_t[:, kc * TB:(kc + 1) * TB],
                                 start=False, stop=False); i += 1
            nc.tensor.matmul(pg[:], gate_s[:, 8 * D_H + mc * 128:
                                           8 * D_H + (mc + 1) * 128],
                             cv_bf[:], start=False, stop=True)
            g_s = gpool.tile([128, TB], F32, tag="gs")
            nc.scalar.activation(g_s[:], pg[:], AF.Sigmoid,
                                 bias=gateb_s[:, mc:mc + 1])
            pc = ps_mm.tile([128, TB], F32, tag="mm")
            nc.tensor.matmul(pc[:], rm_s[:, mc * 128:(mc + 1) * 128],
                             vhat[:], start=True, stop=True)
            dd = cpool.tile([128, TB], F32, tag="dd")
            nc.vector.scalar_tensor_tensor(dd[:], pc[:], rmb_s[:, mc:mc + 1],
                                           ht_t[:, mc * TB:(mc + 1) * TB],
                                           ALU.add, ALU.subtract)
            gd = cpool.tile([128, TB], F32, tag="gd")
            nc.gpsimd.tensor_mul(gd[:], g_s[:], dd[:])
            hn = hpool.tile([128, TB], F32, tag="hn")
            nc.gpsimd.tensor_add(hn[:], gd[:], ht_t[:, mc * TB:(mc + 1) * TB])
            hn_outs.append(hn)

        for c in range(4):
            po = ps_tr.tile([128, TB], F32, tag="trA")
            for mc in range(4):
                nc.tensor.transpose(po[:, mc * 128:(mc + 1) * 128],
                                    hn_outs[mc][:, c * 128:(c + 1) * 128],
                                    ident[:])
            ob = opool.tile([128, TB], F32, tag="ob")
            if c % 2 == 0:
                nc.scalar.copy(ob[:], po[:])
            else:
                nc.vector.tensor_copy(ob[:], po[:])
            nc.sync.dma_start(d["hn_d"][b0 + c * 128:b0 + (c + 1) * 128, :],
                              ob[:])

    esB.close()
    es.close()


def _sigmoid(z):
    return 1.0 / (1.0 + np.exp(-z))


def kernel(**inputs):
    global LAST_EXEC_NS, LAST_RESULTS
    import ml_dtypes

    f32 = np.float32
    g = {k: np.asarray(v) for k, v in inputs.items()}
    a = _sigmoid(np.asarray(g["res_logit_alpha"], np.float64)).astype(f32)
    a_vec = np.concatenate([np.full(128, a[0], f32), np.full(128, a[1], f32),
                            np.full(256, a[2], f32)])
    oma = (1.0 - a_vec).astype(f32)
    br = np.concatenate([g["bx0"] + g["bh0"], g["bx1"] + g["bh1"],
                         g["bx2"] + g["bh2"]]).astype(f32)

    wx = np.vstack([g["Wx0"], g["Wx1"], g["Wx2"]]).astype(f32)   # [512, 256]
    wh = np.vstack([g["Wh0"], g["Wh1"], g["Wh2"]]).astype(f32)   # [512, 512]

    C = np.ascontiguousarray
    shared = {
        "wxT": C(wx.T), "whT": C(wh.T),
        "aT": C(g["A_w"].astype(f32).T), "bT": C(g["B_w"].astype(f32).T),
        "uT": C(g["U_w"].astype(f32).T),
        "keyT": C(g["key_w"].astype(f32).T),
        "valT": C(g["val_w"].astype(f32).T),
        "gateT": C(g["gate_w"].astype(f32).T).astype(ml_dtypes.bfloat16),
        "rmT": C(g["rm_w"].astype(f32).T),
        "ktT": C(g["buf_keys"].astype(f32).T),
        "v0": C(g["buf_vals"].astype(f32)),
        "hebb45": C(0.5 * HEBB_DECAY * g["hebb_M"].astype(f32)),
        "br": br, "oma": oma,
        "ub": g["U_b"].astype(f32), "keyb": g["key_b"].astype(f32),
        "valb": g["val_b"].astype(f32), "gateb": g["gate_b"].astype(f32),
        "rmb": g["rm_b"].astype(f32),
    }
    x = g["x"].astype(f32)
    hp = g["h_prev"].astype(f32)
    ar0 = a[0] * g["r0"].astype(f32)
    ar1 = a[1] * g["r1"].astype(f32)
    ar2 = a[2] * g["r2"].astype(f32)

    in_maps = []
    for i in range(NCORES):
        s = slice(i * BC, (i + 1) * BC)
        m = dict(shared)
        m["x"] = C(x[s])
        m["h_prev"] = C(hp[s])
        m["ar0"] = C(ar0[s])
        m["ar1"] = C(ar1[s])
        m["ar2"] = C(ar2[s])
        in_maps.append(m)

    nc = _build()
    res = run_bass_kernel_spmd(nc, in_maps, list(range(NCORES)), trace=TRACE)
    LAST_EXEC_NS = res.exec_time_ns
    LAST_RESULTS = res

    h_new = np.concatenate([res.results[i]["h_new"] for i in range(NCORES)], 0)
    r0o = np.concatenate([res.results[i]["r0o"] for i in range(NCORES)], 0)
    r1o = np.concatenate([res.results[i]["r1o"] for i in range(NCORES)], 0)
    r2o = np.concatenate([res.results[i]["r2o"] for i in range(NCORES)], 0)
    return (h_new, r0o, r1o, r2o)


# revision 10
# speedup vs baseline: 1.1723x; 1.1723x over previous
"""Trainium2 Bass kernel for nn_CRSDCell_71339406786971.

kernel(**inputs) takes the FULL (unsharded) numpy inputs and returns the full
(h_new, r0_out, r1_out, r2_out) tuple. Internally: data-parallel shard of the
batch dim across 8 NeuronCores, replicated weights, on-chip AllReduce of the
batch-mean key/value and the Hebbian outer product.
"""
import sys
import numpy as np

try:
    import concourse.bass as bass  # noqa: F401
except Exception:
    sys.path.insert(0, "/opt/trn_rl_repo")

import concourse.bass as bass  # noqa: F811
import concourse.mybir as mybir
import concourse.tile as tile
from concourse import bacc
from concourse.bass_utils import run_bass_kernel_spmd
from concourse.masks import make_identity

F32 = mybir.dt.float32
F32R = mybir.dt.float32r
BF16 = mybir.dt.bfloat16
AF = mybir.ActivationFunctionType
ALU = mybir.AluOpType

NCORES = 8
B = 65536
BC = B // NCORES          # 8192 rows per core
TB = 512                  # batch tile (free dim per matmul)
NT = BC // TB             # 16 tiles per core
D_X, D_H = 256, 512
RES = 512                 # total reservoir dim (128+128+256)
D_K = D_V = 64
SLOTS = 64
HEBB_DECAY, HEBB_ETA = 0.9, 0.1

# module-level knobs for the test harness
TRACE = False
LAST_EXEC_NS = None
LAST_RESULTS = None

_AR_GROUPS = [list(range(NCORES))]


def _build():
    nc = bacc.Bacc("TRN2", target_bir_lowering=False, debug=False,
                   num_devices=NCORES)

    d = {}
    # ---- DRAM I/O ----
    d["x_d"] = nc.dram_tensor("x", [BC, D_X], F32, kind="ExternalInput")
    d["h_d"] = nc.dram_tensor("h_prev", [BC, D_H], F32, kind="ExternalInput")
    # r inputs are pre-scaled by a=sigmoid(res_logit_alpha) on host
    d["r0_d"] = nc.dram_tensor("ar0", [BC, 128], F32, kind="ExternalInput")
    d["r1_d"] = nc.dram_tensor("ar1", [BC, 128], F32, kind="ExternalInput")
    d["r2_d"] = nc.dram_tensor("ar2", [BC, 256], F32, kind="ExternalInput")

    d["wx_d"] = nc.dram_tensor("wxT", [D_X, RES], F32R, kind="ExternalInput")
    d["wh_d"] = nc.dram_tensor("whT", [D_H, RES], F32R, kind="ExternalInput")
    d["at_d"] = nc.dram_tensor("aT", [D_H, D_H], F32R, kind="ExternalInput")
    d["bt_d"] = nc.dram_tensor("bT", [RES, D_H], F32R, kind="ExternalInput")
    d["ut_d"] = nc.dram_tensor("uT", [D_X, D_H], F32R, kind="ExternalInput")
    d["key_d"] = nc.dram_tensor("keyT", [RES + D_X, D_K], F32R,
                                kind="ExternalInput")
    d["val_d"] = nc.dram_tensor("valT", [D_H, D_V], F32R, kind="ExternalInput")
    d["gate_d"] = nc.dram_tensor("gateT", [D_H + RES + D_K + D_V, D_H], BF16,
                                 kind="ExternalInput")
    d["rm_d"] = nc.dram_tensor("rmT", [D_V, D_H], F32R, kind="ExternalInput")
    d["kt0_d"] = nc.dram_tensor("ktT", [D_K, SLOTS], F32R, kind="ExternalInput")
    d["v0_d"] = nc.dram_tensor("v0", [SLOTS, D_V], F32R, kind="ExternalInput")
    d["hebb_d"] = nc.dram_tensor("hebb45", [D_K, D_V], F32,
                                 kind="ExternalInput")

    d["br_d"] = nc.dram_tensor("br", [RES], F32, kind="ExternalInput")
    d["oma_d"] = nc.dram_tensor("oma", [RES], F32, kind="ExternalInput")
    d["ub_d"] = nc.dram_tensor("ub", [D_H], F32, kind="ExternalInput")
    d["keyb_d"] = nc.dram_tensor("keyb", [D_K], F32, kind="ExternalInput")
    d["valb_d"] = nc.dram_tensor("valb", [D_V], F32, kind="ExternalInput")
    d["gateb_d"] = nc.dram_tensor("gateb", [D_H], F32, kind="ExternalInput")
    d["rmb_d"] = nc.dram_tensor("rmb", [D_H], F32, kind="ExternalInput")

    d["hn_d"] = nc.dram_tensor("h_new", [BC, D_H], F32, kind="ExternalOutput")
    d["r0o_d"] = nc.dram_tensor("r0o", [BC, 128], F32, kind="ExternalOutput")
    d["r1o_d"] = nc.dram_tensor("r1o", [BC, 128], F32, kind="ExternalOutput")
    d["r2o_d"] = nc.dram_tensor("r2o", [BC, 256], F32, kind="ExternalOutput")

    with tile.TileContext(nc) as tc:
        with nc.allow_low_precision("f32r rounding of matmul inputs by design"):
            _emit(nc, tc, d)
    nc.compile()
    return nc


def _emit(nc, tc, d):
    from contextlib import ExitStack
    es = ExitStack()
    # pools that live for the whole kernel
    wpool = es.enter_context(tc.tile_pool(name="w", bufs=1))
    kpool = es.enter_context(tc.tile_pool(name="kres", bufs=1))
    opool = es.enter_context(tc.tile_pool(name="obm", bufs=3))
    ps_tr = es.enter_context(tc.tile_pool(name="ptr", bufs=3, space="PSUM"))
    ps_mm = es.enter_context(tc.tile_pool(name="pmm", bufs=2, space="PSUM"))
    ps_kv = es.enter_context(tc.tile_pool(name="pkv", bufs=1, space="PSUM"))
    ps_b = es.enter_context(tc.tile_pool(name="pb", bufs=2, space="PSUM"))
    dpool = es.enter_context(tc.tile_pool(name="dram", bufs=1, space="DRAM"))

    # ---- weights into SBUF ----
    def ldw(dram, ktot, m, dt, tag):
        kc = ktot // 128
        t = wpool.tile([128, kc * m], dt, tag=tag)
        nc.sync.dma_start(
            t[:].rearrange("p (c m) -> p c m", c=kc),
            dram[:].rearrange("(c p) m -> p c m", p=128))
        return t

    wx_s = ldw(d["wx_d"], D_X, RES, F32R, "wx")       # [128, 2*512]
    wh_s = ldw(d["wh_d"], D_H, RES, F32R, "wh")       # [128, 4*512]
    at_s = ldw(d["at_d"], D_H, D_H, F32R, "at")
    bt_s = ldw(d["bt_d"], RES, D_H, F32R, "bt")
    ut_s = ldw(d["ut_d"], D_X, D_H, F32R, "ut")
    key_s = ldw(d["key_d"], RES + D_X, D_K, F32R, "key")   # [128, 6*64]
    val_s = ldw(d["val_d"], D_H, D_V, F32R, "val")         # [128, 4*64]
    gate_s = ldw(d["gate_d"], D_H + RES + D_K + D_V, D_H, BF16, "gate")
    rm_s = wpool.tile([D_V, D_H], F32R)
    nc.sync.dma_start(rm_s[:], d["rm_d"][:])
    kt0_s = wpool.tile([D_K, SLOTS], F32R)
    nc.sync.dma_start(kt0_s[:], d["kt0_d"][:])
    v0_s = wpool.tile([SLOTS, D_V], F32R)
    nc.sync.dma_start(v0_s[:], d["v0_d"][:])
    hebb_s = wpool.tile([D_K, D_V], F32)
    nc.sync.dma_start(hebb_s[:], d["hebb_d"][:])

    def ldb(dram, n, tag):
        mc = n // 128
        t = wpool.tile([128, mc], F32, tag=tag)
        nc.sync.dma_start(t[:].rearrange("p (c o) -> p c o", o=1),
                          dram[:].rearrange("(c p) -> p c", p=128))
        return t

    br_s = ldb(d["br_d"], RES, "br")
    oma_s = ldb(d["oma_d"], RES, "oma")
    ub_s = ldb(d["ub_d"], D_H, "ub")
    gateb_s = ldb(d["gateb_d"], D_H, "gateb")
    rmb_s = ldb(d["rmb_d"], D_H, "rmb")
    keyb_s = wpool.tile([D_K, 1], F32)
    nc.sync.dma_start(keyb_s[:], d["keyb_d"][:].rearrange("(p o) -> p o", o=1))
    valb_s = wpool.tile([D_V, 1], F32)
    nc.sync.dma_start(valb_s[:], d["valb_d"][:].rearrange("(p o) -> p o", o=1))

    ident = wpool.tile([128, 128], F32)
    make_identity(nc, ident[:])
    ones_f = wpool.tile([64, 64], F32)
    nc.vector.memset(ones_f[:], 1.0)
    ones_r = wpool.tile([64, 64], F32R)
    nc.vector.tensor_copy(ones_r[:], ones_f[:])
    half_f = wpool.tile([1, 64], F32)
    nc.vector.memset(half_f[:], 0.5)
    half_r = wpool.tile([1, 64], F32R)
    nc.vector.tensor_copy(half_r[:], half_f[:])

    # k kept resident across both phases
    k_res = kpool.tile([D_K, BC], F32R)
    k_acc = wpool.tile([D_K, 1], F32)
    v_acc = wpool.tile([D_V, 1], F32)
    nc.vector.memset(k_acc[:], 0.0)
    nc.vector.memset(v_acc[:], 0.0)

    # DRAM staging, one tile per batch-tile for fine-grained A->B deps.
    # Layout [128, fc*TB + col] (feature-chunk major within each tile).
    rfm_ds = [dpool.tile([128, 4 * TB], BF16, tag=f"rfm{t}", name=f"rfm{t}") for t in range(NT)]
    hpfm_ds = [dpool.tile([128, 4 * TB], BF16, tag=f"hpfm{t}", name=f"hpfm{t}") for t in range(NT)]
    htfm_ds = [dpool.tile([128, 4 * TB], F32, tag=f"htfm{t}", name=f"htfm{t}") for t in range(NT)]
    red_in = dpool.tile([66, 64], F32)
    red_out = dpool.tile([66, 64], F32)

    kv_ps = ps_kv.tile([D_K, D_V], F32)         # persistent outer-product acc

    # ================= PHASE A =================
    esA = ExitStack()
    lpool = esA.enter_context(tc.tile_pool(name="loads", bufs=2))
    fmpool = esA.enter_context(tc.tile_pool(name="fm", bufs=2))
    ropool = esA.enter_context(tc.tile_pool(name="rout", bufs=6))
    htpool = esA.enter_context(tc.tile_pool(name="ht", bufs=5))
    apool = esA.enter_context(tc.tile_pool(name="smallA", bufs=2))

    for t in range(NT):
        b0 = t * TB
        # -- loads (batch-major) --
        x_l = lpool.tile([128, 4 * D_X], F32, tag="xl")
        nc.sync.dma_start(
            x_l[:].rearrange("p (c dd) -> p c dd", c=4),
            d["x_d"][b0:b0 + TB].rearrange("(c p) dd -> p c dd", p=128))
        h_l = lpool.tile([128, 4 * D_H], F32, tag="hl")
        nc.sync.dma_start(
            h_l[:].rearrange("p (c dd) -> p c dd", c=4),
            d["h_d"][b0:b0 + TB].rearrange("(c p) dd -> p c dd", p=128))
        r_l = lpool.tile([128, 4 * RES], F32, tag="rl")
        for rd_, off, w in ((d["r0_d"], 0, 128), (d["r1_d"], 128, 128),
                            (d["r2_d"], 256, 256)):
            nc.sync.dma_start(
                r_l[:].rearrange("p (c dd) -> p c dd", c=4)[:, :, off:off + w],
                rd_[b0:b0 + TB].rearrange("(c p) dd -> p c dd", p=128))

        x3 = x_l[:].rearrange("p (c dd) -> p c dd", c=4)
        h3 = h_l[:].rearrange("p (c dd) -> p c dd", c=4)
        r3 = r_l[:].rearrange("p (c dd) -> p c dd", c=4)

        # -- transpose x, h to feature-major (f32r via evac) --
        xT = fmpool.tile([128, 2 * TB], F32R, tag="xT")
        for fc in range(2):
            ps = ps_tr.tile([128, TB], F32, tag="trA")
            for c in range(4):
                nc.tensor.transpose(ps[:, c * 128:(c + 1) * 128],
                                    x3[:, c, fc * 128:(fc + 1) * 128], ident[:])
            if fc == 0:
                nc.scalar.copy(xT[:, fc * TB:(fc + 1) * TB], ps[:])
            else:
                nc.vector.tensor_copy(xT[:, fc * TB:(fc + 1) * TB], ps[:])
        hT = fmpool.tile([128, 4 * TB], F32R, tag="hT")
        for fc in range(4):
            ps = ps_tr.tile([128, TB], F32, tag="trA")
            for c in range(4):
                nc.tensor.transpose(ps[:, c * 128:(c + 1) * 128],
                                    h3[:, c, fc * 128:(fc + 1) * 128], ident[:])
            if fc % 2 == 0:
                nc.scalar.copy(hT[:, fc * TB:(fc + 1) * TB], ps[:])
            else:
                nc.vector.tensor_copy(hT[:, fc * TB:(fc + 1) * TB], ps[:])
        # stage h_prev fm as bf16 (cast on SWDGE dma)
        nc.gpsimd.dma_start(hpfm_ds[t][:], hT[:].bitcast(F32))

        # -- reservoir update per output chunk --
        r_outs = []
        for mc in range(4):
            ps_r = ps_tr.tile([128, TB], F32, tag="trA")
            for c in range(4):
                nc.tensor.transpose(ps_r[:, c * 128:(c + 1) * 128],
                                    r3[:, c, mc * 128:(mc + 1) * 128], ident[:])
            pm = ps_mm.tile([128, TB], F32, tag="mm")
            nmm = 6
            i = 0
            for kc in range(2):
                nc.tensor.matmul(pm[:], wx_s[:, kc * RES + mc * 128:
                                             kc * RES + (mc + 1) * 128],
                                 xT[:, kc * TB:(kc + 1) * TB],
                                 start=(i == 0), stop=(i == nmm - 1)); i += 1
            for kc in range(4):
                nc.tensor.matmul(pm[:], wh_s[:, kc * RES + mc * 128:
                                             kc * RES + (mc + 1) * 128],
                                 hT[:, kc * TB:(kc + 1) * TB],
                                 start=(i == 0), stop=(i == nmm - 1)); i += 1
            th = apool.tile([128, TB], F32, tag="th")
            nc.scalar.activation(th[:], pm[:], AF.Tanh, bias=br_s[:, mc:mc + 1])
            ro = ropool.tile([128, TB], F32R, tag="ro")
            # r_out = (tanh * (1-a)) + a*r_prev  (a*r_prev transposed, in PSUM)
            nc.vector.scalar_tensor_tensor(ro[:], th[:], oma_s[:, mc:mc + 1],
                                           ps_r[:], ALU.mult, ALU.add)
            r_outs.append(ro)
            # stage r_cat fm as bf16
            nc.gpsimd.dma_start(rfm_ds[t][:, mc * TB:(mc + 1) * TB],
                                ro[:].bitcast(F32))

        # -- h_tilde --
        ht_outs = []
        for mc in range(4):
            pm = ps_mm.tile([128, TB], F32, tag="mm")
            nmm = 10
            i = 0
            for kc in range(4):
                nc.tensor.matmul(pm[:], at_s[:, kc * D_H + mc * 128:
                                             kc * D_H + (mc + 1) * 128],
                                 hT[:, kc * TB:(kc + 1) * TB],
                                 start=(i == 0), stop=(i == nmm - 1)); i += 1
            for kc in range(4):
                nc.tensor.matmul(pm[:], bt_s[:, kc * D_H + mc * 128:
                                             kc * D_H + (mc + 1) * 128],
                                 r_outs[kc][:],
                                 start=(i == 0), stop=(i == nmm - 1)); i += 1
            for kc in range(2):
                nc.tensor.matmul(pm[:], ut_s[:, kc * D_H + mc * 128:
                                             kc * D_H + (mc + 1) * 128],
                                 xT[:, kc * TB:(kc + 1) * TB],
                                 start=(i == 0), stop=(i == nmm - 1)); i += 1
            ht = htpool.tile([128, TB], F32R, tag="htt")
            nc.scalar.activation(ht[:], pm[:], AF.Gelu, bias=ub_s[:, mc:mc + 1])
            ht_outs.append(ht)
            nc.sync.dma_start(htfm_ds[t][:, mc * TB:(mc + 1) * TB],
                               ht[:].bitcast(F32))

        # -- k and v (with running batch sums) --
        pk = ps_mm.tile([64, TB], F32, tag="mm")
        for i in range(6):
            rhs = r_outs[i][:] if i < 4 else xT[:, (i - 4) * TB:(i - 3) * TB]
            nc.tensor.matmul(pk[:], key_s[:, i * D_K:(i + 1) * D_K], rhs,
                             start=(i == 0), stop=(i == 5))
        kp = apool.tile([D_K, 1], F32, tag="kp")
        nc.vector.tensor_scalar(k_res[:, b0:b0 + TB], pk[:], keyb_s[:, :], 0.0,
                                ALU.add, ALU.add, accum_out=kp[:])
        nc.vector.tensor_add(k_acc[:], k_acc[:], kp[:])

        pv = ps_mm.tile([64, TB], F32, tag="mm")
        for i in range(4):
            nc.tensor.matmul(pv[:], val_s[:, i * D_V:(i + 1) * D_V],
                             ht_outs[i][:], start=(i == 0), stop=(i == 3))
        v_t = apool.tile([D_V, TB], F32, tag="vt")
        vp = apool.tile([D_V, 1], F32, tag="vp")
        nc.vector.tensor_scalar(v_t[:], pv[:], valb_s[:, :], 0.0,
                                ALU.add, ALU.add, accum_out=vp[:])
        nc.vector.tensor_add(v_acc[:], v_acc[:], vp[:])

        # -- kv outer product (exact fp32) --
        pt = ps_tr.tile([128, TB], F32, tag="trA")
        for i in range(4):
            nc.tensor.transpose(
                pt[:, i * 64:(i + 1) * 64],
                k_res[:, b0 + i * 128:b0 + (i + 1) * 128].bitcast(F32),
                ident[0:64, 0:64])
            nc.tensor.transpose(pt[:, 256 + i * 64:256 + (i + 1) * 64],
                                v_t[:, i * 128:(i + 1) * 128],
                                ident[0:64, 0:64])
        kvT = apool.tile([128, TB], F32, tag="kvT")
        nc.vector.tensor_copy(kvT[:], pt[:])
        for i in range(4):
            nc.tensor.matmul(kv_ps[:], kvT[:, i * 64:(i + 1) * 64],
                             kvT[:, 256 + i * 64:256 + (i + 1) * 64],
                             start=(t == 0 and i == 0),
                             stop=(t == NT - 1 and i == 3))

        # -- res outputs: transpose back to batch-major, store --
        for c in range(4):
            po = ps_tr.tile([128, TB], F32, tag="trA")
            for mc in range(4):
                nc.tensor.transpose(
                    po[:, mc * 128:(mc + 1) * 128],
                    r_outs[mc][:, c * 128:(c + 1) * 128].bitcast(F32),
                    ident[:])
            ob = opool.tile([128, TB], F32, tag="ob")
            if c % 2 == 0:
                nc.scalar.copy(ob[:], po[:])
            else:
                nc.vector.tensor_copy(ob[:], po[:])
            rb = b0 + c * 128
            nc.sync.dma_start(d["r0o_d"][rb:rb + 128, :], ob[:, 0:128])
            nc.sync.dma_start(d["r1o_d"][rb:rb + 128, :], ob[:, 128:256])
            nc.sync.dma_start(d["r2o_d"][rb:rb + 128, :], ob[:, 256:512])

    esA.close()

    # ================= ALLREDUCE =================
    kv_s = wpool.tile([D_K, D_V], F32)
    nc.vector.tensor_copy(kv_s[:], kv_ps[:])
    nc.sync.dma_start(red_in[0:64, :], kv_s[:])
    nc.sync.dma_start(red_in[64:65, :].rearrange("o (p q) -> p (o q)", q=1),
                      k_acc[:])
    nc.sync.dma_start(red_in[65:66, :].rearrange("o (p q) -> p (o q)", q=1),
                      v_acc[:])
    nc.gpsimd.collective_compute("AllReduce", ALU.add,
                                 replica_groups=_AR_GROUPS,
                                 ins=[red_in.opt()], outs=[red_out.opt()])
    kv_r = wpool.tile([D_K, D_V], F32)
    nc.sync.dma_start(kv_r[:], red_out[0:64, :])
    kmean = wpool.tile([D_K, 1], F32)
    nc.sync.dma_start(kmean[:], red_out[64:65, :].rearrange(
        "o (p q) -> p (o q)", q=1))
    vmean = wpool.tile([1, D_V], F32)
    nc.sync.dma_start(vmean[:], red_out[65:66, :])
    # M_half = 0.45*hebb + (0.05/B)*kv_sum
    m_s = wpool.tile([D_K, D_V], F32R)
    nc.vector.scalar_tensor_tensor(m_s[:], kv_r[:], 0.5 * HEBB_ETA / B,
                                   hebb_s[:], ALU.mult, ALU.add)
    # slot-0 key/value = batch means
    nc.vector.tensor_scalar_mul(kt0_s[:, 0:1], kmean[:], 1.0 / B)
    nc.vector.tensor_scalar_mul(v0_s[0:1, :], vmean[:], 1.0 / B)

    # ================= PHASE B =================
    esB = ExitStack()
    bpool = esB.enter_context(tc.tile_pool(name="bload", bufs=2))
    cpool = esB.enter_context(tc.tile_pool(name="smallB", bufs=2))
    gpool = esB.enter_context(tc.tile_pool(name="gs", bufs=2))
    hpool = esB.enter_context(tc.tile_pool(name="hn", bufs=6))

    for t in range(NT):
        b0 = t * TB
        rfm_t = bpool.tile([128, 4 * TB], BF16, tag="rfm")
        nc.sync.dma_start(rfm_t[:], rfm_ds[t][:])
        hpfm_t = bpool.tile([128, 4 * TB], BF16, tag="hpfm")
        nc.sync.dma_start(hpfm_t[:], hpfm_ds[t][:])
        ht_t = bpool.tile([128, 4 * TB], F32, tag="htl")
        nc.sync.dma_start(ht_t[:], htfm_ds[t][:])

        kk = k_res[:, b0:b0 + TB]

        # c = k / ||k||
        ksq = cpool.tile([D_K, TB], F32R, tag="ksq")
        nc.vector.tensor_mul(ksq[:], kk.bitcast(F32), kk.bitcast(F32))
        p_ss = ps_b.tile([1, TB], F32, tag="pb")
        nc.tensor.matmul(p_ss[:], ones_r[:, 0:1], ksq[:], start=True, stop=True)
        nrm = cpool.tile([1, TB], F32, tag="nrm")
        nc.scalar.activation(nrm[:], p_ss[:], AF.Sqrt)
        rn = cpool.tile([1, TB], F32R, tag="rn")
        nc.vector.reciprocal(rn[:], nrm[:])
        p_bc = ps_b.tile([64, TB], F32, tag="pb")
        nc.tensor.matmul(p_bc[:], ones_r[0:1, :], rn[:], start=True, stop=True)
        c_r = cpool.tile([D_K, TB], F32R, tag="cr")
        nc.vector.tensor_mul(c_r[:], kk.bitcast(F32), p_bc[:])
        cv_bf = cpool.tile([128, TB], BF16, tag="cvbf")
        nc.vector.tensor_copy(cv_bf[0:64, :], c_r[:].bitcast(F32))

        # attention over slots (softmax denominators via PE ones-matmul)
        p_lg = ps_b.tile([SLOTS, TB], F32, tag="pb")
        nc.tensor.matmul(p_lg[:], kt0_s[:], c_r[:], start=True, stop=True)
        e_r = cpool.tile([SLOTS, TB], F32R, tag="er")
        nc.scalar.activation(e_r[:], p_lg[:], AF.Exp, scale=0.125)
        p_dn = ps_b.tile([1, TB], F32, tag="pb")
        nc.tensor.matmul(p_dn[:], ones_r[:, 0:1], e_r[:], start=True, stop=True)
        rd = cpool.tile([1, TB], F32R, tag="rd")
        nc.vector.reciprocal(rd[:], p_dn[:])
        p_rb = ps_b.tile([64, TB], F32, tag="pb")
        nc.tensor.matmul(p_rb[:], half_r[:], rd[:], start=True, stop=True)
        # e_n = e * (0.5/denom)  -> normalized halved attention weights
        e_n = cpool.tile([SLOTS, TB], F32R, tag="en")
        nc.vector.tensor_mul(e_n[:], e_r[:].bitcast(F32), p_rb[:])
        # v_hat = V^T @ e_n + M_half^T @ c   (single PSUM accumulation)
        p_v = ps_b.tile([D_V, TB], F32, tag="pb")
        nc.tensor.matmul(p_v[:], v0_s[:], e_n[:], start=True, stop=False)
        nc.tensor.matmul(p_v[:], m_s[:], c_r[:], start=False, stop=True)
        vhat = cpool.tile([D_V, TB], F32R, tag="vhat")
        nc.vector.tensor_copy(vhat[:], p_v[:])
        nc.vector.tensor_copy(cv_bf[64:128, :], vhat[:].bitcast(F32))

        # gate + chi + h_new
        hn_outs = []
        for mc in range(4):
            pg = ps_mm.tile([128, TB], F32, tag="mm")
            i = 0
            for kc in range(4):
                nc.tensor.matmul(pg[:], gate_s[:, kc * D_H + mc * 128:
                                               kc * D_H + (mc + 1) * 128],
                                 hpfm_t[:, kc * TB:(kc + 1) * TB],
                                 start=(i == 0), stop=False); i += 1
            for kc in range(4):
                nc.tensor.matmul(pg[:], gate_s[:, (4 + kc) * D_H + mc * 128:
                                               (4 + kc) * D_H + (mc + 1) * 128],
                                 rfm_t[:, kc * TB:(kc + 1) * TB],
                                 start=False, stop=False); i += 1
            nc.tensor.matmul(pg[:], gate_s[:, 8 * D_H + mc * 128:
                                           8 * D_H + (mc + 1) * 128],
                             cv_bf[:], start=False, stop=True)
            g_s = gpool.tile([128, TB], F32, tag="gs")
            nc.scalar.activation(g_s[:], pg[:], AF.Sigmoid,
                                 bias=gateb_s[:, mc:mc + 1])
            pc = ps_mm.tile([128, TB], F32, tag="mm")
            nc.tensor.matmul(pc[:], rm_s[:, mc * 128:(mc + 1) * 128],
                             vhat[:], start=True, stop=True)
            dd = cpool.tile([128, TB], F32, tag="dd")
            nc.vector.scalar_tensor_tensor(dd[:], pc[:], rmb_s[:, mc:mc + 1],
                                           ht_t[:, mc * TB:(mc + 1) * TB],
                                           ALU.add, ALU.subtract)
            gd = cpool.tile([128, TB], F32, tag="gd")
            nc.vector.tensor_mul(gd[:], g_s[:], dd[:])
            hn = hpool.tile([128, TB], F32, tag="hn")
            nc.vector.tensor_add(hn[:], gd[:], ht_t[:, mc * TB:(mc + 1) * TB])
            hn_outs.append(hn)

        for c in range(4):
            po = ps_tr.tile([128, TB], F32, tag="trA")
            for mc in range(4):
                nc.tensor.transpose(po[:, mc * 128:(mc + 1) * 128],
                                    hn_outs[mc][:, c * 128:(c + 1) * 128],
                                    ident[:])
            ob = opool.tile([128, TB], F32, tag="ob")
            if c % 2 == 0:
                nc.scalar.copy(ob[:], po[:])
            else:
                nc.vector.tensor_copy(ob[:], po[:])
            nc.sync.dma_start(d["hn_d"][b0 + c * 128:b0 + (c + 1) * 128, :],
                              ob[:])

    esB.close()
    es.close()


def _sigmoid(z):
    return 1.0 / (1.0 + np.exp(-z))


def kernel(**inputs):
    global LAST_EXEC_NS, LAST_RESULTS
    import ml_dtypes

    f32 = np.float32
    g = {k: np.asarray(v) for k, v in inputs.items()}
    a = _sigmoid(np.asarray(g["res_logit_alpha"], np.float64)).astype(f32)
    a_vec = np.concatenate([np.full(128, a[0], f32), np.full(128, a[1], f32),
                            np.full(256, a[2], f32)])
    oma = (1.0 - a_vec).astype(f32)
    br = np.concatenate([g["bx0"] + g["bh0"], g["bx1"] + g["bh1"],
                         g["bx2"] + g["bh2"]]).astype(f32)

    wx = np.vstack([g["Wx0"], g["Wx1"], g["Wx2"]]).astype(f32)   # [512, 256]
    wh = np.vstack([g["Wh0"], g["Wh1"], g["Wh2"]]).astype(f32)   # [512, 512]

    C = np.ascontiguousarray
    shared = {
        "wxT": C(wx.T), "whT": C(wh.T),
        "aT": C(g["A_w"].astype(f32).T), "bT": C(g["B_w"].astype(f32).T),
        "uT": C(g["U_w"].astype(f32).T),
        "keyT": C(g["key_w"].astype(f32).T),
        "valT": C(g["val_w"].astype(f32).T),
        "gateT": C(g["gate_w"].astype(f32).T).astype(ml_dtypes.bfloat16),
        "rmT": C(g["rm_w"].astype(f32).T),
        "ktT": C(g["buf_keys"].astype(f32).T),
        "v0": C(g["buf_vals"].astype(f32)),
        "hebb45": C(0.5 * HEBB_DECAY * g["hebb_M"].astype(f32)),
        "br": br, "oma": oma,
        "ub": g["U_b"].astype(f32), "keyb": g["key_b"].astype(f32),
        "valb": g["val_b"].astype(f32), "gateb": g["gate_b"].astype(f32),
        "rmb": g["rm_b"].astype(f32),
    }
    x = g["x"].astype(f32)
    hp = g["h_prev"].astype(f32)
    ar0 = a[0] * g["r0"].astype(f32)
    ar1 = a[1] * g["r1"].astype(f32)
    ar2 = a[2] * g["r2"].astype(f32)

    in_maps = []
    for i in range(NCORES):
        s = slice(i * BC, (i + 1) * BC)
        m = dict(shared)
        m["x"] = C(x[s])
        m["h_prev"] = C(hp[s])
        m["ar0"] = C(ar0[s])
        m["ar1"] = C(ar1[s])
        m["ar2"] = C(ar2[s])
        in_maps.append(m)

    nc = _build()
    res = run_bass_kernel_spmd(nc, in_maps, list(range(NCORES)), trace=TRACE)
    LAST_EXEC_NS = res.exec_time_ns
    LAST_RESULTS = res

    h_new = np.concatenate([res.results[i]["h_new"] for i in range(NCORES)], 0)
    r0o = np.concatenate([res.results[i]["r0o"] for i in range(NCORES)], 0)
    r1o = np.concatenate([res.results[i]["r1o"] for i in range(NCORES)], 0)
    r2o = np.concatenate([res.results[i]["r2o"] for i in range(NCORES)], 0)
    return (h_new, r0o, r1o, r2o)


# revision 11
# speedup vs baseline: 1.1887x; 1.0140x over previous
"""Trainium2 Bass kernel for nn_CRSDCell_71339406786971.

kernel(**inputs) takes the FULL (unsharded) numpy inputs and returns the full
(h_new, r0_out, r1_out, r2_out) tuple. Internally: data-parallel shard of the
batch dim across 8 NeuronCores, replicated weights, on-chip AllReduce of the
batch-mean key/value and the Hebbian outer product.
"""
import sys
import numpy as np

try:
    import concourse.bass as bass  # noqa: F401
except Exception:
    sys.path.insert(0, "/opt/trn_rl_repo")

import concourse.bass as bass  # noqa: F811
import concourse.mybir as mybir
import concourse.tile as tile
from concourse import bacc
from concourse.bass_utils import run_bass_kernel_spmd
from concourse.masks import make_identity

F32 = mybir.dt.float32
F32R = mybir.dt.float32r
BF16 = mybir.dt.bfloat16
AF = mybir.ActivationFunctionType
ALU = mybir.AluOpType

NCORES = 8
B = 65536
BC = B // NCORES          # 8192 rows per core
TB = 512                  # batch tile (free dim per matmul)
NT = BC // TB             # 16 tiles per core
D_X, D_H = 256, 512
RES = 512                 # total reservoir dim (128+128+256)
D_K = D_V = 64
SLOTS = 64
HEBB_DECAY, HEBB_ETA = 0.9, 0.1

# module-level knobs for the test harness
TRACE = False
LAST_EXEC_NS = None
LAST_RESULTS = None

_AR_GROUPS = [list(range(NCORES))]


def _build():
    nc = bacc.Bacc("TRN2", target_bir_lowering=False, debug=False,
                   num_devices=NCORES)

    d = {}
    # ---- DRAM I/O ----
    d["x_d"] = nc.dram_tensor("x", [BC, D_X], F32, kind="ExternalInput")
    d["h_d"] = nc.dram_tensor("h_prev", [BC, D_H], F32, kind="ExternalInput")
    # r inputs are pre-scaled by a=sigmoid(res_logit_alpha) on host
    d["r0_d"] = nc.dram_tensor("ar0", [BC, 128], F32, kind="ExternalInput")
    d["r1_d"] = nc.dram_tensor("ar1", [BC, 128], F32, kind="ExternalInput")
    d["r2_d"] = nc.dram_tensor("ar2", [BC, 256], F32, kind="ExternalInput")

    d["wx_d"] = nc.dram_tensor("wxT", [D_X, RES], F32R, kind="ExternalInput")
    d["wh_d"] = nc.dram_tensor("whT", [D_H, RES], F32R, kind="ExternalInput")
    d["at_d"] = nc.dram_tensor("aT", [D_H, D_H], F32R, kind="ExternalInput")
    d["bt_d"] = nc.dram_tensor("bT", [RES, D_H], F32R, kind="ExternalInput")
    d["ut_d"] = nc.dram_tensor("uT", [D_X, D_H], F32R, kind="ExternalInput")
    d["key_d"] = nc.dram_tensor("keyT", [RES + D_X, D_K], F32R,
                                kind="ExternalInput")
    d["val_d"] = nc.dram_tensor("valT", [D_H, D_V], F32R, kind="ExternalInput")
    d["gate_d"] = nc.dram_tensor("gateT", [D_H + RES + D_K + D_V, D_H], BF16,
                                 kind="ExternalInput")
    d["rm_d"] = nc.dram_tensor("rmT", [D_V, D_H], F32R, kind="ExternalInput")
    d["kt0_d"] = nc.dram_tensor("ktT", [D_K, SLOTS], F32R, kind="ExternalInput")
    d["v0_d"] = nc.dram_tensor("v0", [SLOTS, D_V], F32R, kind="ExternalInput")
    d["hebb_d"] = nc.dram_tensor("hebb45", [D_K, D_V], F32,
                                 kind="ExternalInput")

    d["br_d"] = nc.dram_tensor("br", [RES], F32, kind="ExternalInput")
    d["oma_d"] = nc.dram_tensor("oma", [RES], F32, kind="ExternalInput")
    d["ub_d"] = nc.dram_tensor("ub", [D_H], F32, kind="ExternalInput")
    d["keyb_d"] = nc.dram_tensor("keyb", [D_K], F32, kind="ExternalInput")
    d["valb_d"] = nc.dram_tensor("valb", [D_V], F32, kind="ExternalInput")
    d["gateb_d"] = nc.dram_tensor("gateb", [D_H], F32, kind="ExternalInput")
    d["rmb_d"] = nc.dram_tensor("rmb", [D_H], F32, kind="ExternalInput")

    d["hn_d"] = nc.dram_tensor("h_new", [BC, D_H], F32, kind="ExternalOutput")
    d["r0o_d"] = nc.dram_tensor("r0o", [BC, 128], F32, kind="ExternalOutput")
    d["r1o_d"] = nc.dram_tensor("r1o", [BC, 128], F32, kind="ExternalOutput")
    d["r2o_d"] = nc.dram_tensor("r2o", [BC, 256], F32, kind="ExternalOutput")

    with tile.TileContext(nc) as tc:
        with nc.allow_low_precision("f32r rounding of matmul inputs by design"):
            _emit(nc, tc, d)
    nc.compile()
    return nc


def _emit(nc, tc, d):
    from contextlib import ExitStack
    es = ExitStack()
    # pools that live for the whole kernel
    wpool = es.enter_context(tc.tile_pool(name="w", bufs=1))
    kpool = es.enter_context(tc.tile_pool(name="kres", bufs=1))
    opool = es.enter_context(tc.tile_pool(name="obm", bufs=3))
    ps_tr = es.enter_context(tc.tile_pool(name="ptr", bufs=3, space="PSUM"))
    ps_mm = es.enter_context(tc.tile_pool(name="pmm", bufs=2, space="PSUM"))
    ps_kv = es.enter_context(tc.tile_pool(name="pkv", bufs=1, space="PSUM"))
    ps_b = es.enter_context(tc.tile_pool(name="pb", bufs=2, space="PSUM"))
    dpool = es.enter_context(tc.tile_pool(name="dram", bufs=1, space="DRAM"))

    # ---- weights into SBUF ----
    def ldw(dram, ktot, m, dt, tag):
        kc = ktot // 128
        t = wpool.tile([128, kc * m], dt, tag=tag)
        nc.sync.dma_start(
            t[:].rearrange("p (c m) -> p c m", c=kc),
            dram[:].rearrange("(c p) m -> p c m", p=128))
        return t

    wx_s = ldw(d["wx_d"], D_X, RES, F32R, "wx")       # [128, 2*512]
    wh_s = ldw(d["wh_d"], D_H, RES, F32R, "wh")       # [128, 4*512]
    at_s = ldw(d["at_d"], D_H, D_H, F32R, "at")
    bt_s = ldw(d["bt_d"], RES, D_H, F32R, "bt")
    ut_s = ldw(d["ut_d"], D_X, D_H, F32R, "ut")
    key_s = ldw(d["key_d"], RES + D_X, D_K, F32R, "key")   # [128, 6*64]
    val_s = ldw(d["val_d"], D_H, D_V, F32R, "val")         # [128, 4*64]
    gate_s = ldw(d["gate_d"], D_H + RES + D_K + D_V, D_H, BF16, "gate")
    rm_s = wpool.tile([D_V, D_H], F32R)
    nc.sync.dma_start(rm_s[:], d["rm_d"][:])
    kt0_s = wpool.tile([D_K, SLOTS], F32R)
    nc.sync.dma_start(kt0_s[:], d["kt0_d"][:])
    v0_s = wpool.tile([SLOTS, D_V], F32R)
    nc.sync.dma_start(v0_s[:], d["v0_d"][:])
    hebb_s = wpool.tile([D_K, D_V], F32)
    nc.sync.dma_start(hebb_s[:], d["hebb_d"][:])

    def ldb(dram, n, tag):
        mc = n // 128
        t = wpool.tile([128, mc], F32, tag=tag)
        nc.sync.dma_start(t[:].rearrange("p (c o) -> p c o", o=1),
                          dram[:].rearrange("(c p) -> p c", p=128))
        return t

    br_s = ldb(d["br_d"], RES, "br")
    oma_s = ldb(d["oma_d"], RES, "oma")
    ub_s = ldb(d["ub_d"], D_H, "ub")
    gateb_s = ldb(d["gateb_d"], D_H, "gateb")
    rmb_s = ldb(d["rmb_d"], D_H, "rmb")
    keyb_s = wpool.tile([D_K, 1], F32)
    nc.sync.dma_start(keyb_s[:], d["keyb_d"][:].rearrange("(p o) -> p o", o=1))
    valb_s = wpool.tile([D_V, 1], F32)
    nc.sync.dma_start(valb_s[:], d["valb_d"][:].rearrange("(p o) -> p o", o=1))

    ident = wpool.tile([128, 128], F32)
    make_identity(nc, ident[:])
    ones_f = wpool.tile([64, 64], F32)
    nc.vector.memset(ones_f[:], 1.0)
    ones_r = wpool.tile([64, 64], F32R)
    nc.vector.tensor_copy(ones_r[:], ones_f[:])
    half_f = wpool.tile([1, 64], F32)
    nc.vector.memset(half_f[:], 0.5)
    half_r = wpool.tile([1, 64], F32R)
    nc.vector.tensor_copy(half_r[:], half_f[:])

    # k kept resident across both phases
    k_res = kpool.tile([D_K, BC], F32R)
    k_acc = wpool.tile([D_K, 1], F32)
    v_acc = wpool.tile([D_V, 1], F32)
    nc.vector.memset(k_acc[:], 0.0)
    nc.vector.memset(v_acc[:], 0.0)

    # DRAM staging, one tile per batch-tile for fine-grained A->B deps.
    # Layout [128, fc*TB + col] (feature-chunk major within each tile).
    rfm_ds = [dpool.tile([128, 4 * TB], BF16, tag=f"rfm{t}", name=f"rfm{t}") for t in range(NT)]
    hpfm_ds = [dpool.tile([128, 4 * TB], BF16, tag=f"hpfm{t}", name=f"hpfm{t}") for t in range(NT)]
    htfm_ds = [dpool.tile([128, 4 * TB], F32, tag=f"htfm{t}", name=f"htfm{t}") for t in range(NT)]
    red_in = dpool.tile([66, 64], F32)
    red_out = dpool.tile([66, 64], F32)

    kv_ps = ps_kv.tile([D_K, D_V], F32)         # persistent outer-product acc

    # ================= PHASE A =================
    esA = ExitStack()
    lpool = esA.enter_context(tc.tile_pool(name="loads", bufs=2))
    fmpool = esA.enter_context(tc.tile_pool(name="fm", bufs=2))
    ropool = esA.enter_context(tc.tile_pool(name="rout", bufs=8))
    htpool = esA.enter_context(tc.tile_pool(name="ht", bufs=6))
    apool = esA.enter_context(tc.tile_pool(name="smallA", bufs=2))

    for t in range(NT):
        b0 = t * TB
        # -- loads (batch-major) --
        x_l = lpool.tile([128, 4 * D_X], F32, tag="xl")
        nc.sync.dma_start(
            x_l[:].rearrange("p (c dd) -> p c dd", c=4),
            d["x_d"][b0:b0 + TB].rearrange("(c p) dd -> p c dd", p=128))
        h_l = lpool.tile([128, 4 * D_H], F32, tag="hl")
        nc.sync.dma_start(
            h_l[:].rearrange("p (c dd) -> p c dd", c=4),
            d["h_d"][b0:b0 + TB].rearrange("(c p) dd -> p c dd", p=128))
        r_l = lpool.tile([128, 4 * RES], F32, tag="rl")
        for rd_, off, w in ((d["r0_d"], 0, 128), (d["r1_d"], 128, 128),
                            (d["r2_d"], 256, 256)):
            nc.sync.dma_start(
                r_l[:].rearrange("p (c dd) -> p c dd", c=4)[:, :, off:off + w],
                rd_[b0:b0 + TB].rearrange("(c p) dd -> p c dd", p=128))

        x3 = x_l[:].rearrange("p (c dd) -> p c dd", c=4)
        h3 = h_l[:].rearrange("p (c dd) -> p c dd", c=4)
        r3 = r_l[:].rearrange("p (c dd) -> p c dd", c=4)

        # -- transpose x, h to feature-major (f32r via evac) --
        xT = fmpool.tile([128, 2 * TB], F32R, tag="xT")
        for fc in range(2):
            ps = ps_tr.tile([128, TB], F32, tag="trA")
            for c in range(4):
                nc.tensor.transpose(ps[:, c * 128:(c + 1) * 128],
                                    x3[:, c, fc * 128:(fc + 1) * 128], ident[:])
            if fc == 0:
                nc.scalar.copy(xT[:, fc * TB:(fc + 1) * TB], ps[:])
            else:
                nc.vector.tensor_copy(xT[:, fc * TB:(fc + 1) * TB], ps[:])
        hT = fmpool.tile([128, 4 * TB], F32R, tag="hT")
        for fc in range(4):
            ps = ps_tr.tile([128, TB], F32, tag="trA")
            for c in range(4):
                nc.tensor.transpose(ps[:, c * 128:(c + 1) * 128],
                                    h3[:, c, fc * 128:(fc + 1) * 128], ident[:])
            if fc % 2 == 0:
                nc.scalar.copy(hT[:, fc * TB:(fc + 1) * TB], ps[:])
            else:
                nc.vector.tensor_copy(hT[:, fc * TB:(fc + 1) * TB], ps[:])
        # stage h_prev fm as bf16 (cast on SWDGE dma)
        nc.gpsimd.dma_start(hpfm_ds[t][:], hT[:].bitcast(F32))

        # -- reservoir update per output chunk --
        r_outs = []
        for mc in range(4):
            ps_r = ps_tr.tile([128, TB], F32, tag="trA")
            for c in range(4):
                nc.tensor.transpose(ps_r[:, c * 128:(c + 1) * 128],
                                    r3[:, c, mc * 128:(mc + 1) * 128], ident[:])
            pm = ps_mm.tile([128, TB], F32, tag="mm")
            nmm = 6
            i = 0
            for kc in range(2):
                nc.tensor.matmul(pm[:], wx_s[:, kc * RES + mc * 128:
                                             kc * RES + (mc + 1) * 128],
                                 xT[:, kc * TB:(kc + 1) * TB],
                                 start=(i == 0), stop=(i == nmm - 1)); i += 1
            for kc in range(4):
                nc.tensor.matmul(pm[:], wh_s[:, kc * RES + mc * 128:
                                             kc * RES + (mc + 1) * 128],
                                 hT[:, kc * TB:(kc + 1) * TB],
                                 start=(i == 0), stop=(i == nmm - 1)); i += 1
            th = apool.tile([128, TB], F32, tag="th")
            nc.scalar.activation(th[:], pm[:], AF.Tanh, bias=br_s[:, mc:mc + 1])
            ro = ropool.tile([128, TB], F32R, tag="ro")
            # r_out = (tanh * (1-a)) + a*r_prev  (a*r_prev transposed, in PSUM)
            nc.vector.scalar_tensor_tensor(ro[:], th[:], oma_s[:, mc:mc + 1],
                                           ps_r[:], ALU.mult, ALU.add)
            r_outs.append(ro)
            # stage r_cat fm as bf16
            nc.gpsimd.dma_start(rfm_ds[t][:, mc * TB:(mc + 1) * TB],
                                ro[:].bitcast(F32))

        # -- h_tilde --
        ht_outs = []
        for mc in range(4):
            pm = ps_mm.tile([128, TB], F32, tag="mm")
            nmm = 10
            i = 0
            for kc in range(4):
                nc.tensor.matmul(pm[:], at_s[:, kc * D_H + mc * 128:
                                             kc * D_H + (mc + 1) * 128],
                                 hT[:, kc * TB:(kc + 1) * TB],
                                 start=(i == 0), stop=(i == nmm - 1)); i += 1
            for kc in range(4):
                nc.tensor.matmul(pm[:], bt_s[:, kc * D_H + mc * 128:
                                             kc * D_H + (mc + 1) * 128],
                                 r_outs[kc][:],
                                 start=(i == 0), stop=(i == nmm - 1)); i += 1
            for kc in range(2):
                nc.tensor.matmul(pm[:], ut_s[:, kc * D_H + mc * 128:
                                             kc * D_H + (mc + 1) * 128],
                                 xT[:, kc * TB:(kc + 1) * TB],
                                 start=(i == 0), stop=(i == nmm - 1)); i += 1
            ht = htpool.tile([128, TB], F32R, tag="htt")
            nc.scalar.activation(ht[:], pm[:], AF.Gelu, bias=ub_s[:, mc:mc + 1])
            ht_outs.append(ht)
            nc.sync.dma_start(htfm_ds[t][:, mc * TB:(mc + 1) * TB],
                               ht[:].bitcast(F32))

        # -- k and v (with running batch sums) --
        pk = ps_mm.tile([64, TB], F32, tag="mm")
        for i in range(6):
            rhs = r_outs[i][:] if i < 4 else xT[:, (i - 4) * TB:(i - 3) * TB]
            nc.tensor.matmul(pk[:], key_s[:, i * D_K:(i + 1) * D_K], rhs,
                             start=(i == 0), stop=(i == 5))
        kt = apool.tile([D_K, TB], F32, tag="kt")
        kp = apool.tile([D_K, 1], F32, tag="kp")
        nc.vector.tensor_scalar(kt[:], pk[:], keyb_s[:, :], 0.0,
                                ALU.add, ALU.add, accum_out=kp[:])
        nc.vector.tensor_add(k_acc[:], k_acc[:], kp[:])
        # c = k/||k|| computed here (independent of the collective) and
        # stored in place of k
        ksq = apool.tile([D_K, TB], F32R, tag="ksq")
        nc.vector.tensor_mul(ksq[:], kt[:], kt[:])
        p_ss = ps_b.tile([1, TB], F32, tag="pb")
        nc.tensor.matmul(p_ss[:], ones_r[:, 0:1], ksq[:], start=True, stop=True)
        nrm = apool.tile([1, TB], F32, tag="nrm")
        nc.scalar.activation(nrm[:], p_ss[:], AF.Sqrt)
        rn = apool.tile([1, TB], F32R, tag="rn")
        nc.vector.reciprocal(rn[:], nrm[:])
        p_bc = ps_b.tile([64, TB], F32, tag="pb")
        nc.tensor.matmul(p_bc[:], ones_r[0:1, :], rn[:], start=True, stop=True)
        nc.vector.tensor_mul(k_res[:, b0:b0 + TB], kt[:], p_bc[:])

        pv = ps_mm.tile([64, TB], F32, tag="mm")
        for i in range(4):
            nc.tensor.matmul(pv[:], val_s[:, i * D_V:(i + 1) * D_V],
                             ht_outs[i][:], start=(i == 0), stop=(i == 3))
        v_t = apool.tile([D_V, TB], F32, tag="vt")
        vp = apool.tile([D_V, 1], F32, tag="vp")
        nc.vector.tensor_scalar(v_t[:], pv[:], valb_s[:, :], 0.0,
                                ALU.add, ALU.add, accum_out=vp[:])
        nc.vector.tensor_add(v_acc[:], v_acc[:], vp[:])

        # -- kv outer product (exact fp32) --
        pt = ps_tr.tile([128, TB], F32, tag="trA")
        for i in range(4):
            nc.tensor.transpose(
                pt[:, i * 64:(i + 1) * 64],
                kt[:, i * 128:(i + 1) * 128],
                ident[0:64, 0:64])
            nc.tensor.transpose(pt[:, 256 + i * 64:256 + (i + 1) * 64],
                                v_t[:, i * 128:(i + 1) * 128],
                                ident[0:64, 0:64])
        kvT = apool.tile([128, TB], F32, tag="kvT")
        nc.vector.tensor_copy(kvT[:], pt[:])
        for i in range(4):
            nc.tensor.matmul(kv_ps[:], kvT[:, i * 64:(i + 1) * 64],
                             kvT[:, 256 + i * 64:256 + (i + 1) * 64],
                             start=(t == 0 and i == 0),
                             stop=(t == NT - 1 and i == 3))

        # -- res outputs: transpose back to batch-major, store --
        for c in range(4):
            po = ps_tr.tile([128, TB], F32, tag="trA")
            for mc in range(4):
                nc.tensor.transpose(
                    po[:, mc * 128:(mc + 1) * 128],
                    r_outs[mc][:, c * 128:(c + 1) * 128].bitcast(F32),
                    ident[:])
            ob = opool.tile([128, TB], F32, tag="ob")
            if c % 2 == 0:
                nc.scalar.copy(ob[:], po[:])
            else:
                nc.vector.tensor_copy(ob[:], po[:])
            rb = b0 + c * 128
            nc.sync.dma_start(d["r0o_d"][rb:rb + 128, :], ob[:, 0:128])
            nc.sync.dma_start(d["r1o_d"][rb:rb + 128, :], ob[:, 128:256])
            nc.sync.dma_start(d["r2o_d"][rb:rb + 128, :], ob[:, 256:512])

    esA.close()

    # ================= ALLREDUCE =================
    kv_s = wpool.tile([D_K, D_V], F32)
    nc.vector.tensor_copy(kv_s[:], kv_ps[:])
    nc.sync.dma_start(red_in[0:64, :], kv_s[:])
    nc.sync.dma_start(red_in[64:65, :].rearrange("o (p q) -> p (o q)", q=1),
                      k_acc[:])
    nc.sync.dma_start(red_in[65:66, :].rearrange("o (p q) -> p (o q)", q=1),
                      v_acc[:])
    nc.gpsimd.collective_compute("AllReduce", ALU.add,
                                 replica_groups=_AR_GROUPS,
                                 ins=[red_in.opt()], outs=[red_out.opt()])
    kv_r = wpool.tile([D_K, D_V], F32)
    nc.sync.dma_start(kv_r[:], red_out[0:64, :])
    kmean = wpool.tile([D_K, 1], F32)
    nc.sync.dma_start(kmean[:], red_out[64:65, :].rearrange(
        "o (p q) -> p (o q)", q=1))
    vmean = wpool.tile([1, D_V], F32)
    nc.sync.dma_start(vmean[:], red_out[65:66, :])
    # M_half = 0.45*hebb + (0.05/B)*kv_sum
    m_s = wpool.tile([D_K, D_V], F32R)
    nc.vector.scalar_tensor_tensor(m_s[:], kv_r[:], 0.5 * HEBB_ETA / B,
                                   hebb_s[:], ALU.mult, ALU.add)
    # slot-0 key/value = batch means
    nc.vector.tensor_scalar_mul(kt0_s[:, 0:1], kmean[:], 1.0 / B)
    nc.vector.tensor_scalar_mul(v0_s[0:1, :], vmean[:], 1.0 / B)

    # ================= PHASE B =================
    esB = ExitStack()
    bpool = esB.enter_context(tc.tile_pool(name="bload", bufs=2))
    cpool = esB.enter_context(tc.tile_pool(name="smallB", bufs=2))
    gpool = esB.enter_context(tc.tile_pool(name="gs", bufs=2))
    hpool = esB.enter_context(tc.tile_pool(name="hn", bufs=6))

    for t in range(NT):
        b0 = t * TB
        rfm_t = bpool.tile([128, 4 * TB], BF16, tag="rfm")
        nc.sync.dma_start(rfm_t[:], rfm_ds[t][:])
        hpfm_t = bpool.tile([128, 4 * TB], BF16, tag="hpfm")
        nc.sync.dma_start(hpfm_t[:], hpfm_ds[t][:])
        ht_t = bpool.tile([128, 4 * TB], F32, tag="htl")
        nc.sync.dma_start(ht_t[:], htfm_ds[t][:])

        c_r = k_res[:, b0:b0 + TB]
        cv_bf = cpool.tile([128, TB], BF16, tag="cvbf")
        nc.vector.tensor_copy(cv_bf[0:64, :], c_r.bitcast(F32))

        # attention over slots (softmax denominators via PE ones-matmul)
        p_lg = ps_b.tile([SLOTS, TB], F32, tag="pb")
        nc.tensor.matmul(p_lg[:], kt0_s[:], c_r, start=True, stop=True)
        e_r = cpool.tile([SLOTS, TB], F32R, tag="er")
        nc.scalar.activation(e_r[:], p_lg[:], AF.Exp, scale=0.125)
        p_dn = ps_b.tile([1, TB], F32, tag="pb")
        nc.tensor.matmul(p_dn[:], ones_r[:, 0:1], e_r[:], start=True, stop=True)
        rd = cpool.tile([1, TB], F32R, tag="rd")
        nc.vector.reciprocal(rd[:], p_dn[:])
        p_rb = ps_b.tile([64, TB], F32, tag="pb")
        nc.tensor.matmul(p_rb[:], half_r[:], rd[:], start=True, stop=True)
        # e_n = e * (0.5/denom)  -> normalized halved attention weights
        e_n = cpool.tile([SLOTS, TB], F32R, tag="en")
        nc.vector.tensor_mul(e_n[:], e_r[:].bitcast(F32), p_rb[:])
        # v_hat = V^T @ e_n + M_half^T @ c   (single PSUM accumulation)
        p_v = ps_b.tile([D_V, TB], F32, tag="pb")
        nc.tensor.matmul(p_v[:], v0_s[:], e_n[:], start=True, stop=False)
        nc.tensor.matmul(p_v[:], m_s[:], c_r, start=False, stop=True)
        vhat = cpool.tile([D_V, TB], F32R, tag="vhat")
        nc.vector.tensor_copy(vhat[:], p_v[:])
        nc.vector.tensor_copy(cv_bf[64:128, :], vhat[:].bitcast(F32))

        # gate + chi + h_new
        hn_outs = []
        for mc in range(4):
            pg = ps_mm.tile([128, TB], F32, tag="mm")
            i = 0
            for kc in range(4):
                nc.tensor.matmul(pg[:], gate_s[:, kc * D_H + mc * 128:
                                               kc * D_H + (mc + 1) * 128],
                                 hpfm_t[:, kc * TB:(kc + 1) * TB],
                                 start=(i == 0), stop=False); i += 1
            for kc in range(4):
                nc.tensor.matmul(pg[:], gate_s[:, (4 + kc) * D_H + mc * 128:
                                               (4 + kc) * D_H + (mc + 1) * 128],
                                 rfm_t[:, kc * TB:(kc + 1) * TB],
                                 start=False, stop=False); i += 1
            nc.tensor.matmul(pg[:], gate_s[:, 8 * D_H + mc * 128:
                                           8 * D_H + (mc + 1) * 128],
                             cv_bf[:], start=False, stop=True)
            g_s = gpool.tile([128, TB], F32, tag="gs")
            nc.scalar.activation(g_s[:], pg[:], AF.Sigmoid,
                                 bias=gateb_s[:, mc:mc + 1])
            pc = ps_mm.tile([128, TB], F32, tag="mm")
            nc.tensor.matmul(pc[:], rm_s[:, mc * 128:(mc + 1) * 128],
                             vhat[:], start=True, stop=True)
            dd = cpool.tile([128, TB], F32, tag="dd")
            nc.vector.scalar_tensor_tensor(dd[:], pc[:], rmb_s[:, mc:mc + 1],
                                           ht_t[:, mc * TB:(mc + 1) * TB],
                                           ALU.add, ALU.subtract)
            gd = cpool.tile([128, TB], F32, tag="gd")
            nc.vector.tensor_mul(gd[:], g_s[:], dd[:])
            hn = hpool.tile([128, TB], F32, tag="hn")
            nc.vector.tensor_add(hn[:], gd[:], ht_t[:, mc * TB:(mc + 1) * TB])
            hn_outs.append(hn)

        for c in range(4):
            po = ps_tr.tile([128, TB], F32, tag="trA")
            for mc in range(4):
                nc.tensor.transpose(po[:, mc * 128:(mc + 1) * 128],
                                    hn_outs[mc][:, c * 128:(c + 1) * 128],
                                    ident[:])
            ob = opool.tile([128, TB], F32, tag="ob")
            if c % 2 == 0:
                nc.scalar.copy(ob[:], po[:])
            else:
                nc.vector.tensor_copy(ob[:], po[:])
            nc.sync.dma_start(d["hn_d"][b0 + c * 128:b0 + (c + 1) * 128, :],
                              ob[:])

    esB.close()
    es.close()


def _sigmoid(z):
    return 1.0 / (1.0 + np.exp(-z))


def kernel(**inputs):
    global LAST_EXEC_NS, LAST_RESULTS
    import ml_dtypes

    f32 = np.float32
    g = {k: np.asarray(v) for k, v in inputs.items()}
    a = _sigmoid(np.asarray(g["res_logit_alpha"], np.float64)).astype(f32)
    a_vec = np.concatenate([np.full(128, a[0], f32), np.full(128, a[1], f32),
                            np.full(256, a[2], f32)])
    oma = (1.0 - a_vec).astype(f32)
    br = np.concatenate([g["bx0"] + g["bh0"], g["bx1"] + g["bh1"],
                         g["bx2"] + g["bh2"]]).astype(f32)

    wx = np.vstack([g["Wx0"], g["Wx1"], g["Wx2"]]).astype(f32)   # [512, 256]
    wh = np.vstack([g["Wh0"], g["Wh1"], g["Wh2"]]).astype(f32)   # [512, 512]

    C = np.ascontiguousarray
    shared = {
        "wxT": C(wx.T), "whT": C(wh.T),
        "aT": C(g["A_w"].astype(f32).T), "bT": C(g["B_w"].astype(f32).T),
        "uT": C(g["U_w"].astype(f32).T),
        "keyT": C(g["key_w"].astype(f32).T),
        "valT": C(g["val_w"].astype(f32).T),
        "gateT": C(g["gate_w"].astype(f32).T).astype(ml_dtypes.bfloat16),
        "rmT": C(g["rm_w"].astype(f32).T),
        "ktT": C(g["buf_keys"].astype(f32).T),
        "v0": C(g["buf_vals"].astype(f32)),
        "hebb45": C(0.5 * HEBB_DECAY * g["hebb_M"].astype(f32)),
        "br": br, "oma": oma,
        "ub": g["U_b"].astype(f32), "keyb": g["key_b"].astype(f32),
        "valb": g["val_b"].astype(f32), "gateb": g["gate_b"].astype(f32),
        "rmb": g["rm_b"].astype(f32),
    }
    x = g["x"].astype(f32)
    hp = g["h_prev"].astype(f32)
    ar0 = a[0] * g["r0"].astype(f32)
    ar1 = a[1] * g["r1"].astype(f32)
    ar2 = a[2] * g["r2"].astype(f32)

    in_maps = []
    for i in range(NCORES):
        s = slice(i * BC, (i + 1) * BC)
        m = dict(shared)
        m["x"] = C(x[s])
        m["h_prev"] = C(hp[s])
        m["ar0"] = C(ar0[s])
        m["ar1"] = C(ar1[s])
        m["ar2"] = C(ar2[s])
        in_maps.append(m)

    nc = _build()
    res = run_bass_kernel_spmd(nc, in_maps, list(range(NCORES)), trace=TRACE)
    LAST_EXEC_NS = res.exec_time_ns
    LAST_RESULTS = res

    h_new = np.concatenate([res.results[i]["h_new"] for i in range(NCORES)], 0)
    r0o = np.concatenate([res.results[i]["r0o"] for i in range(NCORES)], 0)
    r1o = np.concatenate([res.results[i]["r1o"] for i in range(NCORES)], 0)
    r2o = np.concatenate([res.results[i]["r2o"] for i in range(NCORES)], 0)
    return (h_new, r0o, r1o, r2o)
